# revision 1
# baseline (speedup 1.0000x reference)
"""Grouped GEMM (MoE expert layers) on 8 Trainium2 NeuronCores.

Problem: output[s_e:e_e] = input[s_e:e_e] @ weight[e].T for 8 experts with
token counts given by expert_offsets; input [16384, 2048] f32,
weight [8, 5632, 2048] f32.

Strategy: tensor-parallel over out_features. Core c computes ALL tokens
against its contiguous 704-wide slice of OUT. The expert segmentation enters
the program only as trace-time loop bounds, which are identical on every
core, so one SPMD program serves all 8 cores. The host pre-transposes x
(-> [IN, T]) and the per-core weight slice (-> [E, IN, 704]) and converts
both to bf16: halves the dominant x-replication HBM traffic (134->67 MB per
core) and lets LDWEIGHTS use fast-weight-load. The 704-wide w stream per
(token-tile, k-tile) runs as one N=512 matmul (full PSUM bank) plus one
N=192; bf16 streams at 1 row/cycle at any N. Accumulation and output stay
fp32 (rel err ~1.6e-3 from the bf16 inputs, well under tolerance).
"""
import numpy as np
import ml_dtypes

E, IN, OUT, T, NCORES = 8, 2048, 5632, 16384, 8
OUT_C = OUT // NCORES          # 704 out-features per core
P = 128                        # partitions
KT = IN // P                   # 16 k-tiles of 128
NSPLIT = 512                   # full-PSUM-bank chunk of OUT_C
NREM = OUT_C - NSPLIT          # 192-wide remainder chunk
TT_CHUNK = 2                   # token tiles (128 tokens) per x DMA
KTH = KT // 2                  # w DMA split: kt halves for finer deps


def _pad_segments(offsets):
    """Per-expert token counts padded to multiples of P.

    Returns (sizes, padded_sizes, pad_total).
    """
    sizes = np.diff(offsets).astype(int)
    padded = [(-(-s // P)) * P for s in sizes]
    return list(sizes), padded, int(sum(padded))


def _build_program(padded_sizes, dt_in):
    import concourse.bass as bass
    import concourse.mybir as mybir
    from concourse.tile import TileContext
    from wait_legalize_embed import legalize_waits

    Tp = sum(padded_sizes)
    nc = bass.Bass()
    xT_d = nc.dram_tensor("xT", [IN, Tp], dt_in, kind="ExternalInput")
    wT_d = nc.dram_tensor("wT", [E, IN, OUT_C], dt_in, kind="ExternalInput")
    out_d = nc.dram_tensor("out", [Tp, OUT_C], dt_in, kind="ExternalOutput")

    xT_r = xT_d.rearrange("(kt p) t -> p kt t", p=P)

    with TileContext(nc) as tc:
        with tc.tile_pool(name="wpool", bufs=4) as wpool, \
             tc.tile_pool(name="xpool", bufs=6) as xpool, \
             tc.tile_pool(name="opool", bufs=4) as opool, \
             tc.tile_pool(name="ppool", bufs=8, space="PSUM") as ppool:
            tile_base = 0
            ntiles_total = sum(padded_sizes) // P
            for e in range(E):
                ntiles = padded_sizes[e] // P
                if ntiles == 0:
                    continue
                # Weights split into kt halves so the first matmul of an
                # expert waits on only half the expert slice. Expert 0 loads
                # on the scalar HWDGE queue (parallel with x0 on sync) to cut
                # startup; later experts load on sync ahead of their x chunks
                # (the x-pool WAR rotation keeps that queue several chunks
                # ahead of compute, so they arrive prefetched). Stores on the
                # scalar queue wait on PSUM evictions and would throttle any
                # w prefetch placed behind them.
                wT_e = wT_d[e].rearrange("(kt p) n -> p kt n", p=P)
                first = tile_base == 0
                if first:
                    # Quarter-granular w tiles on the scalar queue: the first
                    # matmul of the program waits on only ~0.7 MB.
                    KTQ = KT // 4
                    wq = [wpool.tile([P, KTQ, OUT_C], dt_in, tag=f"wq{q}",
                                     bufs=1, name=f"wq{q}")
                          for q in range(4)]
                    for q in range(4):
                        nc.scalar.dma_start(
                            out=wq[q][:], in_=wT_e[:, q * KTQ : (q + 1) * KTQ, :]
                        )
                    wmap = lambda kt: (wq[kt // KTQ], kt % KTQ)
                else:
                    w_lo = wpool.tile([P, KTH, OUT_C], dt_in, tag="wlo", bufs=2)
                    w_hi = wpool.tile([P, KTH, OUT_C], dt_in, tag="whi", bufs=2)
                    nc.sync.dma_start(out=w_lo[:], in_=wT_e[:, 0:KTH, :])
                    nc.sync.dma_start(out=w_hi[:], in_=wT_e[:, KTH:KT, :])
                    wmap = lambda kt: (w_lo, kt) if kt < KTH else (w_hi, kt - KTH)
                for tt0 in range(0, ntiles, TT_CHUNK):
                    cur = min(TT_CHUNK, ntiles - tt0)
                    t0 = (tile_base + tt0) * P
                    if first and tt0 == 0:
                        # kt-halved x tiles: the first matmul waits ~0.5 MB.
                        xa = xpool.tile([P, KTH, TT_CHUNK * P], dt_in, tag="xa", bufs=1)
                        xb = xpool.tile([P, KTH, TT_CHUNK * P], dt_in, tag="xb", bufs=1)
                        nc.sync.dma_start(
                            out=xa[:, :, : cur * P],
                            in_=xT_r[:, 0:KTH, t0 : t0 + cur * P],
                        )
                        nc.sync.dma_start(
                            out=xb[:, :, : cur * P],
                            in_=xT_r[:, KTH:KT, t0 : t0 + cur * P],
                        )
                        xmap = lambda kt: (xa, kt) if kt < KTH else (xb, kt - KTH)
                    else:
                        x_sb = xpool.tile([P, KT, TT_CHUNK * P], dt_in, tag="x")
                        nc.sync.dma_start(
                            out=x_sb[:, :, : cur * P],
                            in_=xT_r[:, :, t0 : t0 + cur * P],
                        )
                        xmap = lambda kt: (x_sb, kt)
                    # The very last chunk stores per-tile so the final
                    # (serial) store transfer+receipt covers half the bytes
                    # and the first half overlaps the last tile's compute.
                    last_chunk = tile_base + tt0 + cur == ntiles_total
                    o_sb = opool.tile([P, TT_CHUNK, OUT_C], dt_in, tag="o")
                    for j in range(cur):
                        ps0 = ppool.tile([P, NSPLIT], mybir.dt.float32, tag="ps")
                        ps1 = ppool.tile([P, NREM], mybir.dt.float32, tag="ps")
                        for kt in range(KT):
                            x_t, xk = xmap(kt)
                            lhsT = x_t[:, xk, j * P : (j + 1) * P]
                            w_sb, wk = wmap(kt)
                            nc.tensor.matmul(
                                ps0[:], lhsT, w_sb[:, wk, 0:NSPLIT],
                                start=(kt == 0), stop=(kt == KT - 1),
                            )
                            nc.tensor.matmul(
                                ps1[:], lhsT, w_sb[:, wk, NSPLIT:OUT_C],
                                start=(kt == 0), stop=(kt == KT - 1),
                            )
                        nc.vector.tensor_copy(o_sb[:, j, 0:NSPLIT], ps0[:])
                        nc.vector.tensor_copy(o_sb[:, j, NSPLIT:OUT_C], ps1[:])
                        if last_chunk:
                            row = t0 + j * P
                            nc.scalar.dma_start(
                                out=out_d[row : row + P, :], in_=o_sb[:, j, :]
                            )
                    if not last_chunk:
                        # One batched store per x chunk: out rows
                        # t0..t0+cur*128, row (j*128 + p) <- o_sb[p, j, :].
                        nc.scalar.dma_start(
                            out=out_d[t0 : t0 + cur * P, :].rearrange(
                                "(j p) n -> p j n", p=P
                            ),
                            in_=o_sb[:, :cur, :],
                        )
                tile_base += ntiles
    legalize_waits(nc)
    return nc


def _prepare(input, weight, expert_offsets):
    offs = np.asarray(expert_offsets).astype(np.int64)
    sizes, padded_sizes, Tp = _pad_segments(offs)
    x = np.asarray(input, dtype=np.float32)
    w = np.asarray(weight, dtype=np.float32)

    if Tp == T and all(s == p for s, p in zip(sizes, padded_sizes)):
        xT = np.ascontiguousarray(x.T)
    else:
        xp = np.zeros((Tp, IN), dtype=np.float32)
        base = 0
        for e in range(E):
            s, sz = int(offs[e]), sizes[e]
            xp[base : base + sz] = x[s : s + sz]
            base += padded_sizes[e]
        xT = np.ascontiguousarray(xp.T)
    xT = xT.astype(ml_dtypes.bfloat16)

    in_maps = []
    for c in range(NCORES):
        wTc = np.ascontiguousarray(
            w[:, c * OUT_C : (c + 1) * OUT_C, :].transpose(0, 2, 1)
        ).astype(ml_dtypes.bfloat16)
        in_maps.append({"xT": xT, "wT": wTc})
    return sizes, padded_sizes, Tp, in_maps


def _gather(results, sizes, padded_sizes):
    full = np.concatenate(
        [np.asarray(r["out"], dtype=np.float32) for r in results], axis=1
    )
    if sum(sizes) == full.shape[0]:
        return full
    out = np.empty((sum(sizes), OUT), dtype=np.float32)
    base_p = base = 0
    for e in range(E):
        out[base : base + sizes[e]] = full[base_p : base_p + sizes[e]]
        base += sizes[e]
        base_p += padded_sizes[e]
    return out


def run(input, weight, expert_offsets, trace=False):
    import concourse.mybir as mybir
    from concourse.bass_utils import run_bass_kernel_spmd

    sizes, padded_sizes, Tp, in_maps = _prepare(input, weight, expert_offsets)
    nc = _build_program(padded_sizes, mybir.dt.bfloat16)
    core_ids = list(range(NCORES))
    res = run_bass_kernel_spmd(nc, in_maps, core_ids, trace=trace)
    out = _gather(res.results, sizes, padded_sizes)
    return out, res


def kernel(input, weight, expert_offsets):
    out, _ = run(input, weight, expert_offsets)
    return out


# --- embedded helper (kernel.py must be self-contained) ---------------------
import sys as _sys
import types as _types

_wl_src = '''
import concourse.mybir as mybir


def legalize_waits(nc, maxw: int = 1) -> int:
    """Walrus accepts a limited number of sync-wait commands per instruction;
    split extras onto preceding same-engine NOPs (one wait each)."""
    split = 0
    for f in nc.m.functions:
        for blk in f.blocks:
            new_instructions = []
            for inst in blk.instructions:
                si = inst.sync_info
                waits = list(si.on_wait) if si and si.on_wait else []
                if len(waits) > maxw:
                    keep = waits[-maxw:]
                    extra = waits[:-maxw]
                    for w in extra:
                        nop = mybir.InstNoOp(
                            name=nc.get_next_instruction_name(),
                            sync_info=mybir.SyncInfo(on_wait=[w], on_update=[]),
                            bass_nofuse=True,
                            engine=inst.engine,
                        )
                        new_instructions.append(nop)
                        split += 1
                    inst.sync_info = mybir.SyncInfo(
                        on_wait=keep,
                        on_update=list(si.on_update) if si.on_update else [],
                    )
                new_instructions.append(inst)
            blk.instructions = new_instructions
    return split
'''

_wl_mod = _types.ModuleType("wait_legalize_embed")
exec(_wl_src, _wl_mod.__dict__)
_sys.modules["wait_legalize_embed"] = _wl_mod



# revision 2
# speedup vs baseline: 1.0039x; 1.0039x over previous
"""Grouped GEMM (MoE expert layers) on 8 Trainium2 NeuronCores.

Problem: output[s_e:e_e] = input[s_e:e_e] @ weight[e].T for 8 experts with
token counts given by expert_offsets; input [16384, 2048] f32,
weight [8, 5632, 2048] f32.

Strategy: tensor-parallel over out_features. Core c computes ALL tokens
against its contiguous 704-wide slice of OUT. The expert segmentation enters
the program only as trace-time loop bounds, which are identical on every
core, so one SPMD program serves all 8 cores. The host pre-packs x into
chunk-contiguous tiles [NBLK, 128, KT, 256] and the per-core weight slice
into [E, 128, KT, 704], both bf16, so every DMA moves >=5.6KB contiguous
per partition (line-rate descriptors; the previous strided layout paid
512B/1408B descriptors and a ~12us slow first load). Output is likewise
stored chunk-tiled and un-tiled on the host.

A block of 12 zero-matmuls issues at program start (no data deps beyond an
on-chip memset) so the PE HAM clock-gate warms to 2.4 GHz during the
framework preamble + first DMA window instead of running the first ~14us of
real matmuls at 1.2 GHz.

Per (token-tile, k-tile) the 704-wide w stream runs as one N=512 matmul
(full PSUM bank) plus one N=192; bf16 streams at 1 row/cycle at any N.
Accumulation stays fp32 in PSUM; output is written bf16 (rel err ~2.6e-3).
"""
import numpy as np
import ml_dtypes

E, IN, OUT, T, NCORES = 8, 2048, 5632, 16384, 8
OUT_C = OUT // NCORES          # 704 out-features per core
P = 128                        # partitions
KT = IN // P                   # 16 k-tiles of 128
NSPLIT = 512                   # full-PSUM-bank chunk of OUT_C
NREM = OUT_C - NSPLIT          # 192-wide remainder chunk
TT_CHUNK = 2                   # token tiles (128 tokens) per x DMA
CT = TT_CHUNK * P              # 256 tokens per chunk
KTH = KT // 2                  # w DMA split: kt halves for finer deps
NWARM = 12                     # zero-matmuls to warm the PE clock gate


def _pad_segments(offsets):
    """Per-expert token counts padded to multiples of P.

    Returns (sizes, padded_sizes, pad_total).
    """
    sizes = np.diff(offsets).astype(int)
    padded = [(-(-s // P)) * P for s in sizes]
    return list(sizes), padded, int(sum(padded))


def _blocks_of(padded_sizes):
    """Chunk list [(tile_base, cur_tiles), ...] in program order."""
    blocks = []
    tile_base = 0
    for e in range(E):
        ntiles = padded_sizes[e] // P
        for tt0 in range(0, ntiles, TT_CHUNK):
            blocks.append((e, tile_base + tt0, min(TT_CHUNK, ntiles - tt0)))
        tile_base += ntiles
    return blocks


def _build_program(padded_sizes, dt_in):
    import concourse.bass as bass
    import concourse.mybir as mybir
    from concourse.tile import TileContext
    from wait_legalize_embed import legalize_waits

    blocks = _blocks_of(padded_sizes)
    nblk = len(blocks)
    nc = bass.Bass()
    xq_d = nc.dram_tensor("xq", [nblk, P, KT, CT], dt_in, kind="ExternalInput")
    wq_d = nc.dram_tensor("wq", [E, P, KT, OUT_C], dt_in, kind="ExternalInput")
    out_d = nc.dram_tensor(
        "out", [nblk, P, TT_CHUNK, OUT_C], dt_in, kind="ExternalOutput"
    )

    with TileContext(nc) as tc:
        with tc.tile_pool(name="wpool", bufs=4) as wpool, \
             tc.tile_pool(name="xpool", bufs=6) as xpool, \
             tc.tile_pool(name="opool", bufs=4) as opool, \
             tc.tile_pool(name="zpool", bufs=1) as zpool, \
             tc.tile_pool(name="ppool", bufs=8, space="PSUM") as ppool:
            # --- PE warmup: ~5us of dependency-free zero matmuls so the HAM
            # clock gate reaches 8/8 (2.4 GHz) while the first real DMAs are
            # still in flight. Results land in rotating psum slots and are
            # never read.
            wz = zpool.tile([P, NSPLIT], dt_in, tag="wz")
            nc.gpsimd.memset(wz[:], 0.0)
            for _ in range(NWARM):
                pw = ppool.tile([P, NSPLIT], mybir.dt.float32, tag="ps")
                nc.tensor.matmul(pw[:], wz[:, 0:P], wz[:], start=True, stop=True)

            cur_e = -1
            for bi, (e, tbase, cur) in enumerate(blocks):
                if e != cur_e:
                    cur_e = e
                    wT_e = wq_d[e]
                    first = bi == 0
                    if first:
                        # Quarter-granular w tiles on the scalar queue: the
                        # first matmul of the program waits on only ~0.7 MB.
                        KTQ = KT // 4
                        wq = [wpool.tile([P, KTQ, OUT_C], dt_in, tag=f"wq{q}",
                                         bufs=1, name=f"wq{q}")
                              for q in range(4)]
                        for q in range(4):
                            nc.scalar.dma_start(
                                out=wq[q][:],
                                in_=wT_e[:, q * KTQ : (q + 1) * KTQ, :],
                            )
                        wmap = lambda kt: (wq[kt // KTQ], kt % KTQ)
                    else:
                        # Later experts load kt-halves on the sync queue ahead
                        # of their x chunks (x-pool WAR rotation keeps that
                        # queue several chunks ahead of compute).
                        w_lo = wpool.tile([P, KTH, OUT_C], dt_in, tag="wlo", bufs=2)
                        w_hi = wpool.tile([P, KTH, OUT_C], dt_in, tag="whi", bufs=2)
                        nc.sync.dma_start(out=w_lo[:], in_=wT_e[:, 0:KTH, :])
                        nc.sync.dma_start(out=w_hi[:], in_=wT_e[:, KTH:KT, :])
                        wmap = (lambda kt: (w_lo, kt) if kt < KTH
                                else (w_hi, kt - KTH))
                x_sb = xpool.tile([P, KT, CT], dt_in, tag="x")
                nc.sync.dma_start(out=x_sb[:], in_=xq_d[bi])
                # The very last chunk stores per-tile so the final (serial)
                # store transfer+receipt covers half the bytes and the first
                # half overlaps the last tile's compute.
                last_chunk = bi == nblk - 1
                o_sb = opool.tile([P, TT_CHUNK, OUT_C], dt_in, tag="o")
                for j in range(cur):
                    ps0 = ppool.tile([P, NSPLIT], mybir.dt.float32, tag="ps")
                    ps1 = ppool.tile([P, NREM], mybir.dt.float32, tag="ps")
                    for kt in range(KT):
                        lhsT = x_sb[:, kt, j * P : (j + 1) * P]
                        w_sb, wk = wmap(kt)
                        nc.tensor.matmul(
                            ps0[:], lhsT, w_sb[:, wk, 0:NSPLIT],
                            start=(kt == 0), stop=(kt == KT - 1),
                        )
                        nc.tensor.matmul(
                            ps1[:], lhsT, w_sb[:, wk, NSPLIT:OUT_C],
                            start=(kt == 0), stop=(kt == KT - 1),
                        )
                    nc.vector.tensor_copy(o_sb[:, j, 0:NSPLIT], ps0[:])
                    nc.vector.tensor_copy(o_sb[:, j, NSPLIT:OUT_C], ps1[:])
                    if last_chunk:
                        nc.scalar.dma_start(
                            out=out_d[bi, :, j, :], in_=o_sb[:, j, :]
                        )
                if not last_chunk:
                    nc.scalar.dma_start(
                        out=out_d[bi, :, :cur, :], in_=o_sb[:, :cur, :]
                    )
    legalize_waits(nc)
    return nc


def _prepare(input, weight, expert_offsets):
    offs = np.asarray(expert_offsets).astype(np.int64)
    sizes, padded_sizes, Tp = _pad_segments(offs)
    x = np.asarray(input, dtype=np.float32)
    w = np.asarray(weight, dtype=np.float32)

    if Tp == T and all(s == p for s, p in zip(sizes, padded_sizes)):
        xp = x
    else:
        xp = np.zeros((Tp, IN), dtype=np.float32)
        base = 0
        for e in range(E):
            s, sz = int(offs[e]), sizes[e]
            xp[base : base + sz] = x[s : s + sz]
            base += padded_sizes[e]
        xp = np.ascontiguousarray(xp)

    blocks = _blocks_of(padded_sizes)
    nblk = len(blocks)
    # xq[b, p, kt, t] = xp[tok0_b + t, kt*P + p], zero-padded to CT tokens.
    if all(cur == TT_CHUNK for _, _, cur in blocks):
        xq = np.ascontiguousarray(
            xp.astype(ml_dtypes.bfloat16)
            .reshape(nblk, CT, KT, P)
            .transpose(0, 3, 2, 1)
        )
    else:
        xq = np.zeros((nblk, P, KT, CT), dtype=ml_dtypes.bfloat16)
        for b, (_, tbase, cur) in enumerate(blocks):
            blk = xp[tbase * P : (tbase + cur) * P].astype(ml_dtypes.bfloat16)
            xq[b, :, :, : cur * P] = blk.reshape(cur * P, KT, P).transpose(2, 1, 0)

    in_maps = []
    for c in range(NCORES):
        # wq[e, p, kt, n] = w[e, c*OUT_C + n, kt*P + p]
        wqc = np.ascontiguousarray(
            w[:, c * OUT_C : (c + 1) * OUT_C, :]
            .reshape(E, OUT_C, KT, P)
            .transpose(0, 3, 2, 1)
            .astype(ml_dtypes.bfloat16)
        )
        in_maps.append({"xq": xq, "wq": wqc})
    return sizes, padded_sizes, Tp, in_maps


def _gather(results, sizes, padded_sizes):
    blocks = _blocks_of(padded_sizes)
    nblk = len(blocks)
    Tp = sum(padded_sizes)
    # out_tiled [nblk, P, TT_CHUNK, OUT_C]: token (b*CT + j*P + p) is row
    # [p, j] of block b.
    full = np.concatenate(
        [
            np.asarray(r["out"], dtype=np.float32)
            .transpose(0, 2, 1, 3)
            .reshape(nblk * CT, OUT_C)
            for r in results
        ],
        axis=1,
    )
    if sum(sizes) == Tp == nblk * CT:
        return full
    # General path: blocks may be partially filled; re-index per block.
    rows = np.empty((Tp, OUT), dtype=np.float32)
    for b, (_, tbase, cur) in enumerate(blocks):
        rows[tbase * P : (tbase + cur) * P] = full[b * CT : b * CT + cur * P]
    out = np.empty((sum(sizes), OUT), dtype=np.float32)
    base_p = base = 0
    for e in range(E):
        out[base : base + sizes[e]] = rows[base_p : base_p + sizes[e]]
        base += sizes[e]
        base_p += padded_sizes[e]
    return out


def run(input, weight, expert_offsets, trace=False):
    import concourse.mybir as mybir
    from concourse.bass_utils import run_bass_kernel_spmd

    sizes, padded_sizes, Tp, in_maps = _prepare(input, weight, expert_offsets)
    nc = _build_program(padded_sizes, mybir.dt.bfloat16)
    core_ids = list(range(NCORES))
    res = run_bass_kernel_spmd(nc, in_maps, core_ids, trace=trace)
    out = _gather(res.results, sizes, padded_sizes)
    return out, res


def kernel(input, weight, expert_offsets):
    out, _ = run(input, weight, expert_offsets)
    return out


# --- embedded helper (kernel.py must be self-contained) ---------------------
import sys as _sys
import types as _types

_wl_src = '''
import concourse.mybir as mybir


def legalize_waits(nc, maxw: int = 1) -> int:
    """Walrus accepts a limited number of sync-wait commands per instruction;
    split extras onto preceding same-engine NOPs (one wait each)."""
    split = 0
    for f in nc.m.functions:
        for blk in f.blocks:
            new_instructions = []
            for inst in blk.instructions:
                si = inst.sync_info
                waits = list(si.on_wait) if si and si.on_wait else []
                if len(waits) > maxw:
                    keep = waits[-maxw:]
                    extra = waits[:-maxw]
                    for w in extra:
                        nop = mybir.InstNoOp(
                            name=nc.get_next_instruction_name(),
                            sync_info=mybir.SyncInfo(on_wait=[w], on_update=[]),
                            bass_nofuse=True,
                            engine=inst.engine,
                        )
                        new_instructions.append(nop)
                        split += 1
                    inst.sync_info = mybir.SyncInfo(
                        on_wait=keep,
                        on_update=list(si.on_update) if si.on_update else [],
                    )
                new_instructions.append(inst)
            blk.instructions = new_instructions
    return split
'''

_wl_mod = _types.ModuleType("wait_legalize_embed")
exec(_wl_src, _wl_mod.__dict__)
_sys.modules["wait_legalize_embed"] = _wl_mod


# revision 11
# speedup vs baseline: 1.0779x; 1.0737x over previous
"""Grouped GEMM (MoE expert layers) on 8 Trainium2 NeuronCores.

Problem: output[s_e:e_e] = input[s_e:e_e] @ weight[e].T for 8 experts with
token counts given by expert_offsets; input [16384, 2048] f32,
weight [8, 5632, 2048] f32.

Strategy: tensor-parallel over out_features. Core c computes ALL tokens
against its contiguous 704-wide slice of OUT. The expert segmentation enters
the program only as trace-time loop bounds, identical on every core, so one
SPMD program serves all 8 cores.

Precision: 14 of 16 k-tiles run bf16; the last 2 k-tiles run as one fp8
(e4m3) DoubleRow matmul pair at 2 rows/cycle. Power-of-2 scale folding
(x/8, w*8) keeps the fp8 operands in e4m3's normal range with the product
scale exactly 1, so fp8 partials accumulate into the same PSUM group as the
bf16 partials. Host-simulated rel err ~1.36e-2 (gate 2e-2); pure-bf16
measures 2.6e-3 on HW.

Data movement: host pre-packs x/w into chunk-contiguous DRAM tiles so every
DMA moves multi-KB contiguous runs per partition. The sync queue carries
only x; all w and output stores ride the scalar queue. Experts 0-1 load w
in 2-ktile pieces (~0.36 MB) so first-token compute starts as soon as the
first piece lands; experts >=2 prefetch kt-halves one expert ahead. A short
block of zero-matmuls at program start warms the PE HAM clock gate to
2.4 GHz during the framework preamble; stalls thereafter stay under the
~3.4 us HAM idle window so the PE never re-throttles.
"""
import numpy as np
import ml_dtypes

E, IN, OUT, T, NCORES = 8, 2048, 5632, 16384, 8
OUT_C = OUT // NCORES          # 704 out-features per core
P = 128                        # partitions
KT = IN // P                   # 16 k-tiles of 128
KT8 = 2                        # k-tiles computed in fp8 DoubleRow
KTB = KT - KT8                 # 14 k-tiles computed in bf16
K8SCALE = 8.0                  # x/8, w*8 scale fold for e4m3 range
NSPLIT = 512                   # full-PSUM-bank chunk of OUT_C
NREM = OUT_C - NSPLIT          # 192-wide remainder chunk
TT_CHUNK = 2                   # token tiles (128 tokens) per x DMA
CT = TT_CHUNK * P              # 256 tokens per chunk
WPIECE = 2                     # bf16 k-tiles per w DMA for experts 0-1
KTH = KTB // 2                 # 7: kt-half split for later experts
NWARM = 8                      # zero-matmuls to warm the PE clock gate


def _pad_segments(offsets):
    """Per-expert token counts padded to multiples of P."""
    sizes = np.diff(offsets).astype(int)
    padded = [(-(-s // P)) * P for s in sizes]
    return list(sizes), padded, int(sum(padded))


def _blocks_of(padded_sizes):
    """Chunk list [(expert, tile_base, cur_tiles), ...] in program order."""
    blocks = []
    tile_base = 0
    for e in range(E):
        ntiles = padded_sizes[e] // P
        for tt0 in range(0, ntiles, TT_CHUNK):
            blocks.append((e, tile_base + tt0, min(TT_CHUNK, ntiles - tt0)))
        tile_base += ntiles
    return blocks


def _build_program(padded_sizes, dt_in):
    import concourse.bass as bass
    import concourse.mybir as mybir
    from concourse.tile import TileContext
    from wait_legalize_embed import legalize_waits

    dt8 = mybir.dt.float8e4
    blocks = _blocks_of(padded_sizes)
    nblk = len(blocks)
    experts = sorted({e for e, _, _ in blocks})  # experts with tokens
    nexp = len(experts)
    nc = bass.Bass()
    xq_d = nc.dram_tensor("xq", [nblk, P, KTB, CT], dt_in, kind="ExternalInput")
    x8_d = nc.dram_tensor("x8", [nblk, P, KT8, CT], dt8, kind="ExternalInput")
    wq_d = nc.dram_tensor("wq", [E, P, KTB, OUT_C], dt_in, kind="ExternalInput")
    w8_d = nc.dram_tensor("w8", [E, P, KT8, OUT_C], dt8, kind="ExternalInput")
    out_d = nc.dram_tensor(
        "out", [nblk, P, TT_CHUNK, OUT_C], dt_in, kind="ExternalOutput"
    )

    NPIECE = KTB // WPIECE  # 7 w pieces for fine-grained experts

    with TileContext(nc) as tc:
        with tc.tile_pool(name="wpool", bufs=2) as wpool, \
             tc.tile_pool(name="w8pool", bufs=3) as w8pool, \
             tc.tile_pool(name="xpool", bufs=4) as xpool, \
             tc.tile_pool(name="x8pool", bufs=4) as x8pool, \
             tc.tile_pool(name="opool", bufs=5) as opool, \
             tc.tile_pool(name="zpool", bufs=1) as zpool, \
             tc.tile_pool(name="ppool", bufs=8, space="PSUM") as ppool:
            # --- PE warmup: ~3.4us of dependency-free zero matmuls so the
            # HAM clock gate reaches 8/8 (2.4 GHz) during the framework
            # preamble + first DMA window. Results are never read.
            wz = zpool.tile([P, NSPLIT], dt_in, tag="wz")
            nc.gpsimd.memset(wz[:], 0.0)
            for _ in range(NWARM):
                pw = ppool.tile([P, NSPLIT], mybir.dt.float32, tag="ps")
                nc.tensor.matmul(pw[:], wz[:, 0:P], wz[:], start=True, stop=True)

            # --- w loading. All w rides the scalar queue (sync is x-only so
            # an x-pool WAR wait never head-of-line-blocks a w transfer).
            # Experts 0-1 (the startup bandwidth crunch) load in 2-ktile
            # pieces; later experts load kt-halves, issued one expert ahead.
            wtiles = {}

            def load_expert_fine(e):
                ps = [wpool.tile([P, WPIECE, OUT_C], dt_in, tag=f"wp{i}",
                                 bufs=2, name=f"wp{i}") for i in range(NPIECE)]
                for i in range(NPIECE):
                    nc.scalar.dma_start(
                        out=ps[i][:], in_=wq_d[e][:, i * WPIECE:(i + 1) * WPIECE, :]
                    )
                w8t = w8pool.tile([P, KT8, OUT_C], dt8, tag="w8", name="w8f")
                nc.scalar.dma_start(out=w8t[:], in_=w8_d[e][:])
                wtiles[e] = (
                    lambda kt, ps=ps: (ps[kt // WPIECE], kt % WPIECE), w8t
                )

            def load_expert_half(e):
                w_lo = wpool.tile([P, KTH, OUT_C], dt_in, tag="wlo", bufs=3,
                                  name="wlo")
                w_hi = wpool.tile([P, KTH, OUT_C], dt_in, tag="whi", bufs=3,
                                  name="whi")
                nc.scalar.dma_start(out=w_lo[:], in_=wq_d[e][:, 0:KTH, :])
                nc.scalar.dma_start(out=w_hi[:], in_=wq_d[e][:, KTH:KTB, :])
                w8t = w8pool.tile([P, KT8, OUT_C], dt8, tag="w8", name="w8h")
                nc.scalar.dma_start(out=w8t[:], in_=w8_d[e][:])
                wtiles[e] = (
                    lambda kt, lo=w_lo, hi=w_hi: (lo, kt) if kt < KTH
                    else (hi, kt - KTH),
                    w8t,
                )

            cur_e = -1
            for bi, (e, tbase, cur) in enumerate(blocks):
                if e != cur_e:
                    cur_e = e
                    ei = experts.index(e)
                    if bi == 0:
                        load_expert_fine(experts[0])
                        if nexp > 1:
                            load_expert_fine(experts[1])
                        if nexp > 2:
                            load_expert_half(experts[2])
                    # Prefetch coarse experts two experts ahead (expert 2 can
                    # be tiny, so depth-1 would stall its successor).
                    if ei >= 1 and ei + 2 < nexp:
                        load_expert_half(experts[ei + 2])
                    wmap, w8t = wtiles[e]
                x_sb = xpool.tile([P, KTB, CT], dt_in, tag="x")
                if bi == 0:
                    # kt-halved first load: first matmul waits on ~0.45 MB.
                    nc.sync.dma_start(
                        out=x_sb[:, 0:KTH, :], in_=xq_d[bi][:, 0:KTH, :]
                    )
                    nc.sync.dma_start(
                        out=x_sb[:, KTH:KTB, :], in_=xq_d[bi][:, KTH:KTB, :]
                    )
                else:
                    nc.sync.dma_start(out=x_sb[:], in_=xq_d[bi])
                x8_sb = x8pool.tile([P, KT8, CT], dt8, tag="x8")
                nc.sync.dma_start(out=x8_sb[:], in_=x8_d[bi])
                # The very last chunk stores per-tile so the final (serial)
                # store transfer+receipt covers half the bytes and the first
                # half overlaps the last tile's compute.
                last_chunk = bi == nblk - 1
                o_sb = opool.tile([P, TT_CHUNK, OUT_C], dt_in, tag="o")
                for j in range(cur):
                    ps0 = ppool.tile([P, NSPLIT], mybir.dt.float32, tag="ps")
                    ps1 = ppool.tile([P, NREM], mybir.dt.float32, tag="ps")
                    for kt in range(KTB):
                        lhsT = x_sb[:, kt, j * P : (j + 1) * P]
                        w_sb, wk = wmap(kt)
                        nc.tensor.matmul(
                            ps0[:], lhsT, w_sb[:, wk, 0:NSPLIT],
                            start=(kt == 0), stop=False,
                        )
                        nc.tensor.matmul(
                            ps1[:], lhsT, w_sb[:, wk, NSPLIT:OUT_C],
                            start=(kt == 0), stop=False,
                        )
                    # Last 2 k-tiles as one fp8 DoubleRow pair (2 rows/cycle).
                    lhsT8 = x8_sb[:, :, j * P : (j + 1) * P]
                    nc.tensor.matmul(
                        ps0[:], lhsT8, w8t[:, :, 0:NSPLIT],
                        start=False, stop=True,
                        perf_mode=mybir.MatmulPerfMode.DoubleRow,
                    )
                    nc.tensor.matmul(
                        ps1[:], lhsT8, w8t[:, :, NSPLIT:OUT_C],
                        start=False, stop=True,
                        perf_mode=mybir.MatmulPerfMode.DoubleRow,
                    )
                    nc.vector.tensor_copy(o_sb[:, j, 0:NSPLIT], ps0[:])
                    nc.vector.tensor_copy(o_sb[:, j, NSPLIT:OUT_C], ps1[:])
                    if last_chunk:
                        nc.scalar.dma_start(
                            out=out_d[bi, :, j, :], in_=o_sb[:, j, :]
                        )
                if not last_chunk:
                    nc.scalar.dma_start(
                        out=out_d[bi, :, :cur, :], in_=o_sb[:, :cur, :]
                    )
    legalize_waits(nc)
    return nc


def _prepare(input, weight, expert_offsets):
    offs = np.asarray(expert_offsets).astype(np.int64)
    sizes, padded_sizes, Tp = _pad_segments(offs)
    x = np.asarray(input, dtype=np.float32)
    w = np.asarray(weight, dtype=np.float32)

    if Tp == T and all(s == p for s, p in zip(sizes, padded_sizes)):
        xp = x
    else:
        xp = np.zeros((Tp, IN), dtype=np.float32)
        base = 0
        for e in range(E):
            s, sz = int(offs[e]), sizes[e]
            xp[base : base + sz] = x[s : s + sz]
            base += padded_sizes[e]

    blocks = _blocks_of(padded_sizes)
    nblk = len(blocks)
    KB = KTB * P  # 1792: k-range covered in bf16
    # xq[b, p, kt, t] = xp[tok0_b + t, kt*P + p]; x8[b, p, i, t] likewise for
    # the last KT8 k-tiles, scaled 1/K8SCALE into e4m3.
    if all(cur == TT_CHUNK for _, _, cur in blocks):
        x4 = xp.reshape(nblk, CT, KT, P)
        xq = np.ascontiguousarray(
            x4[:, :, :KTB, :].transpose(0, 3, 2, 1).astype(ml_dtypes.bfloat16)
        )
        x8 = np.ascontiguousarray(
            (x4[:, :, KTB:, :].transpose(0, 3, 2, 1) * (1.0 / K8SCALE)).astype(
                ml_dtypes.float8_e4m3
            )
        )
    else:
        xq = np.zeros((nblk, P, KTB, CT), dtype=ml_dtypes.bfloat16)
        x8 = np.zeros((nblk, P, KT8, CT), dtype=ml_dtypes.float8_e4m3)
        for b, (_, tbase, cur) in enumerate(blocks):
            blk = xp[tbase * P : (tbase + cur) * P]  # [cur*P, IN]
            b4 = blk.reshape(cur * P, KT, P)
            xq[b, :, :, : cur * P] = (
                b4[:, :KTB, :].transpose(2, 1, 0).astype(ml_dtypes.bfloat16)
            )
            x8[b, :, :, : cur * P] = (
                b4[:, KTB:, :].transpose(2, 1, 0) * (1.0 / K8SCALE)
            ).astype(ml_dtypes.float8_e4m3)

    in_maps = []
    for c in range(NCORES):
        wsl = w[:, c * OUT_C : (c + 1) * OUT_C, :]  # [E, OUT_C, IN]
        w4 = wsl.reshape(E, OUT_C, KT, P)
        # wq[e, p, kt, n] = w[e, c*OUT_C + n, kt*P + p]
        wqc = np.ascontiguousarray(
            w4[:, :, :KTB, :].transpose(0, 3, 2, 1).astype(ml_dtypes.bfloat16)
        )
        w8c = np.ascontiguousarray(
            (w4[:, :, KTB:, :].transpose(0, 3, 2, 1) * K8SCALE).astype(
                ml_dtypes.float8_e4m3
            )
        )
        in_maps.append({"xq": xq, "x8": x8, "wq": wqc, "w8": w8c})
    return sizes, padded_sizes, Tp, in_maps


def _gather(results, sizes, padded_sizes):
    blocks = _blocks_of(padded_sizes)
    nblk = len(blocks)
    Tp = sum(padded_sizes)
    # out_tiled [nblk, P, TT_CHUNK, OUT_C]: token (b*CT + j*P + p) is row
    # [p, j] of block b.
    full = np.concatenate(
        [
            np.asarray(r["out"], dtype=np.float32)
            .transpose(0, 2, 1, 3)
            .reshape(nblk * CT, OUT_C)
            for r in results
        ],
        axis=1,
    )
    if sum(sizes) == Tp == nblk * CT:
        return full
    rows = np.empty((Tp, OUT), dtype=np.float32)
    for b, (_, tbase, cur) in enumerate(blocks):
        rows[tbase * P : (tbase + cur) * P] = full[b * CT : b * CT + cur * P]
    out = np.empty((sum(sizes), OUT), dtype=np.float32)
    base_p = base = 0
    for e in range(E):
        out[base : base + sizes[e]] = rows[base_p : base_p + sizes[e]]
        base += sizes[e]
        base_p += padded_sizes[e]
    return out


def run(input, weight, expert_offsets, trace=False):
    import concourse.mybir as mybir
    from concourse.bass_utils import run_bass_kernel_spmd

    sizes, padded_sizes, Tp, in_maps = _prepare(input, weight, expert_offsets)
    nc = _build_program(padded_sizes, mybir.dt.bfloat16)
    core_ids = list(range(NCORES))
    res = run_bass_kernel_spmd(nc, in_maps, core_ids, trace=trace)
    out = _gather(res.results, sizes, padded_sizes)
    return out, res


def kernel(input, weight, expert_offsets):
    out, _ = run(input, weight, expert_offsets)
    return out


# --- embedded helper (kernel.py must be self-contained) ---------------------
import sys as _sys
import types as _types

_wl_src = '''
import concourse.mybir as mybir


def legalize_waits(nc, maxw: int = 1) -> int:
    """Walrus accepts a limited number of sync-wait commands per instruction;
    split extras onto preceding same-engine NOPs (one wait each)."""
    split = 0
    for f in nc.m.functions:
        for blk in f.blocks:
            new_instructions = []
            for inst in blk.instructions:
                si = inst.sync_info
                waits = list(si.on_wait) if si and si.on_wait else []
                if len(waits) > maxw:
                    keep = waits[-maxw:]
                    extra = waits[:-maxw]
                    for w in extra:
                        nop = mybir.InstNoOp(
                            name=nc.get_next_instruction_name(),
                            sync_info=mybir.SyncInfo(on_wait=[w], on_update=[]),
                            bass_nofuse=True,
                            engine=inst.engine,
                        )
                        new_instructions.append(nop)
                        split += 1
                    inst.sync_info = mybir.SyncInfo(
                        on_wait=keep,
                        on_update=list(si.on_update) if si.on_update else [],
                    )
                new_instructions.append(inst)
            blk.instructions = new_instructions
    return split
'''

_wl_mod = _types.ModuleType("wait_legalize_embed")
exec(_wl_src, _wl_mod.__dict__)
_sys.modules["wait_legalize_embed"] = _wl_mod
